# revision 1
# baseline (speedup 1.0000x reference)
"""GCN encoder (2-layer GCNConv) on 8 Trainium2 NeuronCores.

Strategy (dest-sharded graph parallel):
  - Destinations sharded by node range across 8 cores (12500 each).
  - Gathers use InstDMAGatherAnt (dma_gather): ~8192 int16 indices per
    instruction (0.34 ns/descriptor on the Pool Q7, vs ~1 us per 128-row
    indirect_dma_start; multi-offset indirect DMA is broken in HW ucode).
    int16 limits a gather to <32768 table rows, so the replicated node table
    is split into 4 sections (2 cores each).
  - Per (core, section) ELL: dests sorted by per-section in-degree k_b
    descending; round t holds the t-th section-b edge of each dest with
    k_b > t, so round t writes a contiguous prefix of that section's
    accumulator and the scatter-add becomes a contiguous DVE tensor_add.
    The 4 per-section accumulators (each in its own dest order) are merged
    into canonical order by four 12544-index dma_gather permutes.
  - dma_gather requires a 256B-multiple source row stride: the bf16 u-tables
    are AllGather'ed COMPACT (100352 x 16, 3.2 MB) and locally expanded to a
    256B-strided copy (strided HWDGE DMA); only 32B/row is read.
  - Both layers aggregate 16-wide features: layer 2 aggregates BEFORE the
    16x64 transform (aggregation commutes with right-multiplication by W2).

out = D^-1/2 (A+I) D^-1/2 relu(D^-1/2 (A+I) D^-1/2 X W1 + b1) W2 + b2
with u = h * dinv:  s[c] = sum_{e: col=c} u[row_e] + u[c];  out_h = s * dinv + b
"""

import math
import sys

import numpy as np

if "/opt/trn_rl_repo" not in sys.path:
    sys.path.insert(0, "/opt/trn_rl_repo")

import concourse.bacc as bacc
import concourse.bass as bass
import concourse.mybir as mybir
import concourse.tile as tile
from concourse import bass_utils
from concourse.masks import make_identity

# ---------------------------------------------------------------- constants
N = 100000
E = 3200000
IN_C, HID, OUT_C = 128, 16, 64
NCORES = 8
SHARD = N // NCORES            # 12500 real dests per core
P = 128
QCH = (SHARD + P - 1) // P     # 98 column-chunks of 128 ranks
SHARD_PAD = QCH * P            # 12544
SFREE = QCH * HID              # 1568 f32 per partition for s
TABLE_ROWS = NCORES * SHARD_PAD  # 100352 rows in the replicated u table
NSEC = 4                       # table sections (int16 gather-index range)
SEC_ROWS = TABLE_ROWS // NSEC  # 25088 rows (2 cores) per section
PAD_DEG = 1.0e30               # huge degree for pad ranks -> dinv ~ 1e-15
GCOLS = 64                     # slot columns per dma_gather (8192 descs)
PADROW = 84 * QCH + 97         # local p-major row of pad rank 12500 (u = 0)
MCOLS = SHARD_PAD // 16        # 784 wrapped idx columns per merge gather

F32 = mybir.dt.float32
BF16 = mybir.dt.bfloat16
I32 = mybir.dt.int32
I16 = mybir.dt.int16


def _table_row_local(rank):
    """p-major row of a rank within its core block (matches the staging DMA
    layout '(p q) f -> p (q f)')."""
    return (rank % P) * QCH + rank // P


def _round_profile_sec():
    """Static fallback per-(section, round) widths; the exact per-input
    profile is computed in prep. Per-section in-degree ~ Poisson(E/N/NSEC)
    with dests sorted by it (tight prefix)."""
    lam_b = E / N / NSEC
    R_MAX = 80
    W = []
    for t in range(R_MAX):
        pmf = math.exp(-lam_b)
        cdf = pmf
        for k in range(1, t + 1):
            pmf = pmf * lam_b / k
            cdf += pmf
        q = max(1.0 - cdf, 0.0)
        if SHARD * q < 1e-9 and t > lam_b:
            break
        nt = SHARD * q
        sig = math.sqrt(max(SHARD * q * (1.0 - q), 0.0))
        w = int(math.ceil((nt + 8.0 * sig + 64.0) / P))
        W.append(max(1, min(QCH, w)))
    W += [1] * 6
    return [list(W) for _ in range(NSEC)]


def _pack_runs(runs, bcols):
    """Pack (local_col, width) runs into blocks of <= bcols columns, splitting
    wide runs. Returns list of (block_col_start, ncols, [(lc, j0, w)...]) with
    lc relative to block start and j0 the piece's chunk offset within its
    round."""
    blocks = []
    cur, c0, curw = [], None, 0
    for lc, w in runs:
        j0 = 0
        while w > 0:
            take = min(w, bcols - curw)
            if c0 is None:
                c0 = lc
            cur.append((lc - c0, j0, take))
            curw += take
            lc += take
            j0 += take
            w -= take
            if curw >= bcols:
                blocks.append((c0, curw, cur))
                cur, c0, curw = [], None, 0
    if cur:
        blocks.append((c0, curw, cur))
    return blocks


def _dma_gather(eng, out_ap, in_ap, idxs_ap, num_idxs, elem_size, elem_step,
                single_packet=True):
    """InstDMAGatherAnt, mirroring bass BassGpSimd.dma_gather but allowing
    elem_size < 256B (the encoding only requires the row STRIDE, elem_step,
    to be a 256B multiple; verified on HW). idxs must be int16, wrapped
    [16, n/16] and replicated across the 8 GPSIMD stripes (128 partitions)."""
    dt_sz = mybir.dt.size(in_ap.dtype)
    stride_bytes = elem_step * dt_sz
    assert stride_bytes % 256 == 0
    _in_ap = eng.lower_ap_dma(in_ap, for_custom_bir_dma=True)
    _idxs_ap = eng.lower_ap(idxs_ap)
    _out_ap = eng.lower_ap(out_ap)
    return eng.add_instruction(
        mybir.InstDMAGatherAnt(
            name=eng.bass.get_next_instruction_name(),
            ins=[*_in_ap, _idxs_ap, eng.lower_val_access(eng.to_reg(num_idxs))],
            outs=[_out_ap],
            transpose=False,
            num_idxs=num_idxs,
            elem_size=elem_size,
            stride_bytes_256=stride_bytes // 256,
            gen_mode=0,
            single_packet=single_packet,
            queue_num=0,
            sbuf_tokens_per_rank=0,
            sbuf_free_dim_per_rank=0,
            sbuf_free_dim_pad_per_rank=0,
            sbuf_byte_offset=0,
        )
    )


# ---------------------------------------------------------------- device code
def _build_program(sec_round_w=None):
    """sec_round_w: list of NSEC lists of per-round column widths."""
    import os
    _skip_ag = bool(os.environ.get("SKIP_AG"))
    _skip_agg = bool(os.environ.get("SKIP_AGG"))
    _skip_merge = bool(os.environ.get("SKIP_MERGE"))
    _skip_out = bool(os.environ.get("SKIP_OUT"))
    if sec_round_w is None:
        sec_round_w = _round_profile_sec()
    sec_cols = [sum(w) for w in sec_round_w]
    sec_runs = []
    for b in range(NSEC):
        runs, pos = [], 0
        for w in sec_round_w[b]:
            runs.append((pos, w))
            pos += w
        sec_runs.append(runs)

    nc = bacc.Bacc(
        "TRN2",
        target_bir_lowering=False,
        debug=False,
        num_devices=NCORES,
        enable_partition_id=False,
        num_swdge_queues=4,
    )
    xT = nc.dram_tensor("xT", [P, SHARD_PAD], BF16, kind="ExternalInput")
    deg_in = nc.dram_tensor("deg", [P, QCH], F32, kind="ExternalInput")
    # wrapped int16 gather indices [16, 8*cols_b per section]; both layers
    # share one copy (identical slot->source mapping)
    offs_in = nc.dram_tensor(
        "offs", [16, 8 * sum(sec_cols)], I16, kind="ExternalInput"
    )
    # wrapped int16 merge-permute indices, MCOLS wrapped cols per section
    midx_in = nc.dram_tensor(
        "midx", [16, NSEC * MCOLS], I16, kind="ExternalInput"
    )
    w1_in = nc.dram_tensor("W1", [IN_C, HID], BF16, kind="ExternalInput")
    w2_in = nc.dram_tensor("W2", [HID, OUT_C], F32, kind="ExternalInput")
    b1_in = nc.dram_tensor("b1", [P, HID], F32, kind="ExternalInput")
    b2_in = nc.dram_tensor("b2", [P, OUT_C], F32, kind="ExternalInput")
    out_d = nc.dram_tensor("out", [SHARD_PAD, OUT_C], F32, kind="ExternalOutput")

    import os as _os
    _gb = int(_os.environ.get("GBUFS", "6"))
    _mb = int(_os.environ.get("MBUFS", "2"))
    with tile.TileContext(nc) as tc:
        with (
            tc.tile_pool(name="const", bufs=1) as cpool,
            tc.tile_pool(name="offs", bufs=2) as opool,
            tc.tile_pool(name="gath", bufs=_gb) as gpool,
            tc.tile_pool(name="merge", bufs=_mb) as mpool,
            tc.tile_pool(name="psum", bufs=3, space="PSUM") as ppool,
            tc.tile_pool(name="psumT", bufs=3, space="PSUM") as ptpool,
            tc.tile_pool(name="dram", bufs=1, space="DRAM") as dpool,
        ):
            # ---- load constants / inputs
            w1_sb = cpool.tile([IN_C, HID], BF16, name="w1_sb")
            w2_sb = cpool.tile([HID, OUT_C], F32, name="w2_sb")
            b1_sb = cpool.tile([P, HID], F32, name="b1_sb")
            b2_sb = cpool.tile([P, OUT_C], F32, name="b2_sb")
            ident = cpool.tile([P, P], F32, name="ident")
            deg_sb = cpool.tile([P, QCH], F32, name="deg_sb")
            dinv = cpool.tile([P, QCH], F32, name="dinv")
            midx_sb = cpool.tile([P, NSEC * MCOLS], I16, name="midx_sb")
            xT_sb = cpool.tile([P, SHARD_PAD], BF16, name="xT_sb")
            u_own = cpool.tile([P, SFREE], BF16, name="u_own")
            u2_own = cpool.tile([P, SFREE], BF16, name="u2_own")
            s_acc = cpool.tile([P, SFREE], F32, name="s_acc")
            v_sb = cpool.tile([P, SFREE], F32, name="v_sb")
            acc = [
                cpool.tile([P, SFREE], F32, name=f"acc{b}") for b in range(NSEC)
            ]
            out_sb = cpool.tile([P, QCH * OUT_C], F32, name="out_sb")

            nc.sync.dma_start(out=w1_sb[:], in_=w1_in[:])
            nc.sync.dma_start(out=w2_sb[:], in_=w2_in[:])
            nc.sync.dma_start(out=b1_sb[:], in_=b1_in[:])
            nc.sync.dma_start(out=b2_sb[:], in_=b2_in[:])
            nc.sync.dma_start(out=deg_sb[:], in_=deg_in[:])
            nc.sync.dma_start(out=xT_sb[:], in_=xT[:])
            msrc = midx_in[:]
            nc.sync.dma_start(
                out=midx_sb[:],
                in_=bass.AP(msrc.tensor, msrc.offset, [[0, 8]] + msrc.ap),
            )
            make_identity(nc, ident[:])

            nc.vector.reciprocal(dinv[:], deg_sb[:])
            nc.scalar.activation(dinv[:], dinv[:], mybir.ActivationFunctionType.Sqrt)

            def dinv16():
                a = dinv[:]
                return bass.AP(a.tensor, a.offset, [a.ap[0], a.ap[1], [0, HID]])

            def b16(t, f):
                a = t[:]
                return bass.AP(a.tensor, a.offset, [a.ap[0], [0, QCH], [1, f]])

            def shaped(ap):
                return ap.rearrange("p (q f) -> p q f", f=HID)

            dram_u1own = dpool.tile([SHARD_PAD, HID], BF16, name="dram_u1own")
            dram_u2own = dpool.tile([SHARD_PAD, HID], BF16, name="dram_u2own")
            u1_tab = dpool.tile(
                [TABLE_ROWS, HID], BF16, name="u1_tab", addr_space="Shared"
            )
            u2_tab = dpool.tile(
                [TABLE_ROWS, HID], BF16, name="u2_tab", addr_space="Shared"
            )
            # 256B-strided gather copies (only first 16 of 128 cols written)
            u1_pad = dpool.tile([TABLE_ROWS, P], BF16, name="u1_pad")
            u2_pad = dpool.tile([TABLE_ROWS, P], BF16, name="u2_pad")
            # 256B-strided per-section accumulator stagings (f32, 64-elem rows)
            dram_acc = [
                [
                    dpool.tile([SHARD_PAD, 64], F32, name=f"dram_acc{li}{b}")
                    for b in range(NSEC)
                ]
                for li in range(2)
            ]

            # ---- layer-1 transform: u1 = (x @ W1) * dinv (bf16), chunk-wise
            for q in range(QCH):
                pt = ppool.tile([P, HID], F32, name="mm1", tag="mm")
                nc.tensor.matmul(
                    out=pt[:],
                    lhsT=xT_sb[:, q * P : (q + 1) * P],
                    rhs=w1_sb[:],
                    start=True,
                    stop=True,
                )
                nc.vector.tensor_scalar(
                    out=u_own[:, q * HID : (q + 1) * HID],
                    in0=pt[:],
                    scalar1=dinv[:, q : q + 1],
                    scalar2=None,
                    op0=mybir.AluOpType.mult,
                )

            def stage_ag_expand(u_sb, dram_own, tab, tab_pad):
                # own slice -> DRAM rows (p-major), AllGather compact table,
                # then per-section strided expand to the 256B-stride copy
                nc.sync.dma_start(
                    out=dram_own[:].rearrange("(p q) f -> p (q f)", p=P),
                    in_=u_sb[:],
                )
                if not _skip_ag:
                    nc.gpsimd.collective_compute(
                        "AllGather",
                        mybir.AluOpType.bypass,
                        replica_groups=[list(range(NCORES))],
                        ins=[dram_own.opt()],
                        outs=[tab.opt()],
                    )
                for b in range(NSEC):
                    r0 = b * SEC_ROWS
                    nc.sync.dma_start(
                        out=tab_pad[r0 : r0 + SEC_ROWS, 0:HID],
                        in_=tab[r0 : r0 + SEC_ROWS, :],
                    )

            stage_ag_expand(u_own, dram_u1own, u1_tab, u1_pad)

            # ---- aggregation of one layer from the padded table
            def aggregate(tab_pad, sacc, li):
                nc.vector.memset(sacc[:], 0.0)
                MW = 512  # wrapped cols per merge sub-gather (8192 idxs)

                def merge(b):
                    if _skip_merge:
                        return
                    # sacc += perm_b(acc_b) via staged-DRAM dma_gather
                    mg = mpool.tile([P, SFREE], F32, name="mg", tag="mg")
                    for m0 in range(0, MCOLS, MW):
                        mw = min(MW, MCOLS - m0)
                        _dma_gather(
                            nc.gpsimd,
                            out_ap=mg[
                                :, (m0 // 8) * HID : ((m0 + mw) // 8) * HID
                            ].rearrange("p (c e) -> p c e", e=HID),
                            in_ap=dram_acc[li][b][:, 0:HID],
                            idxs_ap=midx_sb[
                                :, b * MCOLS + m0 : b * MCOLS + m0 + mw
                            ],
                            num_idxs=mw * 16,
                            elem_size=HID,
                            elem_step=64,
                            single_packet=False,
                        )
                    nc.vector.tensor_tensor(
                        out=sacc[:], in0=sacc[:], in1=mg[:],
                        op=mybir.AluOpType.add,
                    )

                col_base = 0
                for b in range(NSEC):
                    cols_b = sec_cols[b]
                    nc.vector.memset(acc[b][:], 0.0)
                    # stream section idxs: broadcast [16, 8*cols] to 128 parts
                    ob = opool.tile([P, 8 * cols_b], I16, name="ob", tag="ob")
                    src = offs_in[:, 8 * col_base : 8 * (col_base + cols_b)]
                    bsrc = bass.AP(src.tensor, src.offset, [[0, 8]] + src.ap)
                    nc.sync.dma_start(out=ob[:], in_=bsrc)
                    r0 = b * SEC_ROWS
                    for c0, ncols, bruns in (
                        [] if _skip_agg else _pack_runs(sec_runs[b], GCOLS)
                    ):
                        g = gpool.tile(
                            [P, GCOLS * HID], BF16, name="gbuf", tag="gbuf"
                        )
                        _dma_gather(
                            nc.gpsimd,
                            out_ap=g[:, : ncols * HID].rearrange(
                                "p (c e) -> p c e", e=HID
                            ),
                            in_ap=tab_pad[r0 : r0 + SEC_ROWS, 0:HID],
                            idxs_ap=ob[:, 8 * c0 : 8 * (c0 + ncols)],
                            num_idxs=ncols * P,
                            elem_size=HID,
                            elem_step=P,
                            single_packet=False,
                        )
                        for lc, j0, w in bruns:
                            nc.vector.tensor_tensor(
                                out=acc[b][:, j0 * HID : (j0 + w) * HID],
                                in0=acc[b][:, j0 * HID : (j0 + w) * HID],
                                in1=g[:, lc * HID : (lc + w) * HID],
                                op=mybir.AluOpType.add,
                            )
                    col_base += cols_b
                    # stage acc_b to a 256B-strided DRAM table (p-major rows)
                    da = dram_acc[li][b]
                    nc.sync.dma_start(
                        out=bass.AP(
                            da[:].tensor,
                            da[:].offset,
                            [[64 * QCH, P], [64, QCH], [1, HID]],
                        ),
                        in_=acc[b][:],
                    )
                    # merge one section late so its wait never stalls the
                    # Pool queue; merges 0..2 hide under later sections
                    if b >= 1:
                        merge(b - 1)
                merge(NSEC - 1)

            aggregate(u1_pad, s_acc, 0)

            # self loop + finalize: u2 = relu((s + u1) * dinv + b1) * dinv
            nc.vector.tensor_tensor(
                out=s_acc[:], in0=s_acc[:], in1=u_own[:], op=mybir.AluOpType.add
            )
            nc.vector.tensor_tensor(
                out=shaped(s_acc[:]), in0=shaped(s_acc[:]), in1=dinv16(),
                op=mybir.AluOpType.mult,
            )
            nc.vector.tensor_tensor(
                out=shaped(s_acc[:]), in0=shaped(s_acc[:]), in1=b16(b1_sb, HID),
                op=mybir.AluOpType.add,
            )
            nc.scalar.activation(
                s_acc[:], s_acc[:], mybir.ActivationFunctionType.Relu
            )
            nc.vector.tensor_tensor(
                out=shaped(u2_own[:]), in0=shaped(s_acc[:]), in1=dinv16(),
                op=mybir.AluOpType.mult,
            )

            stage_ag_expand(u2_own, dram_u2own, u2_tab, u2_pad)

            # ---- layer-2 aggregation into v, then out = (v*dinv) @ W2 + b2
            aggregate(u2_pad, v_sb, 1)
            nc.vector.tensor_tensor(
                out=v_sb[:], in0=v_sb[:], in1=u2_own[:], op=mybir.AluOpType.add
            )
            nc.vector.tensor_tensor(
                out=shaped(v_sb[:]), in0=shaped(v_sb[:]), in1=dinv16(),
                op=mybir.AluOpType.mult,
            )

            for q in range(0 if not _skip_out else QCH, QCH):
                ptt = ptpool.tile([HID, P], F32, name="vT_ps", tag="vT_ps")
                nc.tensor.transpose(
                    out=ptt[:],
                    in_=v_sb[:, q * HID : (q + 1) * HID],
                    identity=ident[:],
                )
                vT = gpool.tile([HID, P], F32, name="vT_sb", tag="vT_sb")
                nc.vector.tensor_copy(out=vT[:], in_=ptt[:])
                po = ppool.tile([P, OUT_C], F32, name="mm2", tag="mm")
                nc.tensor.matmul(
                    out=po[:], lhsT=vT[:], rhs=w2_sb[:], start=True, stop=True
                )
                nc.vector.tensor_tensor(
                    out=out_sb[:, q * OUT_C : (q + 1) * OUT_C],
                    in0=po[:],
                    in1=b2_sb[:],
                    op=mybir.AluOpType.add,
                )

            nc.sync.dma_start(
                out=out_d[:].rearrange("(p q) f -> p (q f)", p=P),
                in_=out_sb[:],
            )

    nc.compile()
    return nc


_NC_CACHE = {}


def _get_program(sec_round_w=None):
    key = (
        tuple(tuple(w) for w in sec_round_w)
        if sec_round_w is not None
        else None
    )
    if key not in _NC_CACHE:
        _NC_CACHE[key] = _build_program(sec_round_w)
    return _NC_CACHE[key]


# ---------------------------------------------------------------- host prep
def _prep_inputs(x, edge_index, W1, b1, W2, b2):
    """Pure index preprocessing + layout (sharding). Returns in_maps, the
    inverse row permutation for unsharding, and the per-section round
    profile."""
    import ml_dtypes

    x = np.asarray(x, dtype=np.float32)
    row = np.asarray(edge_index[0], dtype=np.int64)
    col = np.asarray(edge_index[1], dtype=np.int64)
    W1 = np.asarray(W1, dtype=np.float32)
    W2 = np.asarray(W2, dtype=np.float32)
    b1 = np.asarray(b1, dtype=np.float32).reshape(-1)
    b2 = np.asarray(b2, dtype=np.float32).reshape(-1)

    indeg = np.bincount(col, minlength=N).astype(np.int64)  # excl self loop
    deg = (indeg + 1).astype(np.float32)

    # canonical per-core rank: own range sorted by total in-degree descending
    rank = np.empty(N, dtype=np.int64)
    node_of_rank = np.empty((NCORES, SHARD_PAD), dtype=np.int64)
    for c in range(NCORES):
        nodes = np.arange(c * SHARD, (c + 1) * SHARD)
        order = np.argsort(-indeg[nodes], kind="stable")
        rank[nodes[order]] = np.arange(SHARD)
        node_of_rank[c, :SHARD] = nodes[order]
        node_of_rank[c, SHARD:] = -1

    core_of = np.arange(N) // SHARD
    # source row within its section's padded table (odd cores upper half)
    local_row = core_of % 2 * SHARD_PAD + _table_row_local(rank)
    sec_of = core_of // 2

    # per-section in-degree per (core, canonical rank)
    dcore_all = col // SHARD
    drank_all = rank[col]
    ssec_all = sec_of[row]
    kb = np.zeros((NSEC, NCORES, SHARD_PAD), dtype=np.int32)
    np.add.at(kb, (ssec_all, dcore_all, drank_all), 1)

    # per-(core, section) dest order: sort by k_b descending; srank = position
    srank = np.empty((NSEC, NCORES, SHARD_PAD), dtype=np.int64)
    sorder = np.empty((NSEC, NCORES, SHARD_PAD), dtype=np.int64)
    for b in range(NSEC):
        for c in range(NCORES):
            o = np.argsort(-kb[b, c], kind="stable")
            sorder[b, c] = o
            srank[b, c, o] = np.arange(SHARD_PAD)

    # exact per-section round profile over the per-section sort (tight):
    # W^b_t = max over cores of ceil(#{k_b > t}/128)
    sec_round_w = []
    for b in range(NSEC):
        maxk = int(kb[b].max())
        wlist = []
        for t in range(maxk):
            wt = 1
            for c in range(NCORES):
                n_tc = int(np.count_nonzero(kb[b, c] > t))
                wt = max(wt, (n_tc + P - 1) // P)
            wlist.append(wt)
        if not wlist:
            wlist = [1]
        sec_round_w.append(wlist)

    sec_cols = [sum(w) for w in sec_round_w]
    tot_cols = sum(sec_cols)
    _prep_inputs.pad_frac = tot_cols * P * NCORES / E - 1.0

    # per-edge slot: section srank of dest + within-(dest,section) counter
    ekey = (dcore_all * SHARD_PAD + drank_all) * NSEC + ssec_all
    eorder = np.argsort(ekey, kind="stable")
    ekey_s = ekey[eorder]
    row_s = row[eorder]
    starts = np.searchsorted(ekey_s, np.arange(NCORES * SHARD_PAD * NSEC))
    t_of = np.arange(E) - starts[ekey_s]
    dsec = ekey_s % NSEC
    drank_s = ekey_s // NSEC % SHARD_PAD
    dc_s = ekey_s // (NSEC * SHARD_PAD)
    sr = srank[dsec, dc_s, drank_s]  # per-section rank of the dest
    qq, pp = sr // P, sr % P

    secbase = np.concatenate([[0], np.cumsum(sec_cols)]).astype(np.int64)
    nr_b = np.asarray([len(w) for w in sec_round_w], dtype=np.int64)
    wt_flat = np.concatenate(
        [np.asarray(w + [0], dtype=np.int64) for w in sec_round_w]
    )
    wbase = np.concatenate([[0], np.cumsum(nr_b + 1)]).astype(np.int64)
    cumw_flat = np.concatenate(
        [np.concatenate([[0], np.cumsum(sec_round_w[b])[:-1]])
         for b in range(NSEC)]
    ).astype(np.int64)
    cb = np.concatenate([[0], np.cumsum(nr_b)]).astype(np.int64)

    tcl = np.minimum(t_of, nr_b[dsec] - 1)
    ok = (t_of < nr_b[dsec]) & (qq < wt_flat[wbase[dsec] + tcl])
    if not np.all(ok):
        raise RuntimeError("per-section round profile exceeded")
    colpos = cumw_flat[cb[dsec] + t_of] + qq      # column within section
    k_flat = (secbase[dsec] + colpos) * P + pp    # global flat slot index
    offs_all = np.full((NCORES, 16, 8 * tot_cols), PADROW, dtype=np.int16)
    offs_all[dc_s, k_flat % 16, k_flat // 16] = local_row[row_s].astype(
        np.int16
    )

    # merge-permute idxs: for canonical rank r, read acc_b at srank[b, c, r]
    midx_all = np.zeros((NCORES, 16, NSEC * MCOLS), dtype=np.int16)
    for b in range(NSEC):
        for c in range(NCORES):
            src_pos = srank[b, c]  # [SHARD_PAD] canonical rank -> srank
            # gather idx k = canonical rank r; table row = p-major of srank
            vals = _table_row_local(src_pos).astype(np.int16)
            k = np.arange(SHARD_PAD)
            midx_all[c, k % 16, b * MCOLS + k // 16] = vals
    # NOTE: gather k -> out[k%128, k//128] = slot (p=r%128, q=r//128) matches
    # s_acc layout (rank r at [r%128, (r//128)*HID]) when k = r.

    # per-core tensors
    in_maps = []
    b1b = np.broadcast_to(b1, (P, HID)).astype(np.float32).copy()
    b2b = np.broadcast_to(b2, (P, OUT_C)).astype(np.float32).copy()
    W1_bf = W1.astype(ml_dtypes.bfloat16)
    for c in range(NCORES):
        nor = node_of_rank[c]
        deg_pi = np.full(SHARD_PAD, PAD_DEG, dtype=np.float32)
        deg_pi[:SHARD] = deg[nor[:SHARD]]
        deg_sb = deg_pi.reshape(QCH, P).T.copy()
        xT = np.zeros((P, SHARD_PAD), dtype=ml_dtypes.bfloat16)
        xT[:, :SHARD] = x[nor[:SHARD]].T.astype(ml_dtypes.bfloat16)
        in_maps.append(
            {
                "xT": np.ascontiguousarray(xT),
                "deg": np.ascontiguousarray(deg_sb),
                "offs": np.ascontiguousarray(offs_all[c]),
                "midx": np.ascontiguousarray(midx_all[c]),
                "W1": W1_bf,
                "W2": W2,
                "b1": b1b,
                "b2": b2b,
            }
        )

    # unshard: out row of node (concat over cores) = core*SHARD_PAD + p-major
    inv_rows = core_of * SHARD_PAD + _table_row_local(rank)
    global OFFS_W
    OFFS_W = tot_cols
    return in_maps, inv_rows, sec_round_w


OFFS_W = 0


def _build_floor_probe():
    """Minimal 8-core program for measuring the PJRT dispatch floor."""
    nc = bacc.Bacc("TRN2", target_bir_lowering=False, debug=False,
                   num_devices=NCORES, enable_partition_id=False)
    a = nc.dram_tensor("a", [P, 16], F32, kind="ExternalInput")
    b = nc.dram_tensor("b", [P, 16], F32, kind="ExternalOutput")
    with tile.TileContext(nc) as tc:
        with tc.tile_pool(name="sb", bufs=1) as sb:
            t = sb.tile([P, 16], F32, name="t")
            nc.sync.dma_start(out=t[:], in_=a[:])
            nc.sync.dma_start(out=b[:], in_=t[:])
    nc.compile()
    return nc


def timed_run(in_maps, reps=5, nc=None, round_w=None):
    """Time device execution of the compiled program (PJRT path, inputs
    pre-staged on device). Returns best wall-ns per execution."""
    import time

    import jax
    from jax.sharding import Mesh, PartitionSpec
    from jax.experimental.shard_map import shard_map as _shard_map

    if nc is None:
        nc = _get_program(round_w)
    import concourse.mybir as _mb
    from concourse.bass2jax import _bass_exec_p, install_neuronx_cc_hook

    install_neuronx_cc_hook()
    in_names, out_names, out_avals, zero_outs = [], [], [], []
    for alloc in nc.m.functions[0].allocations:
        if not isinstance(alloc, _mb.MemoryLocationSet):
            continue
        name = alloc.memorylocations[0].name
        if alloc.kind == "ExternalInput":
            in_names.append(name)
        elif alloc.kind == "ExternalOutput":
            out_names.append(name)
            shape = tuple(alloc.tensor_shape)
            dtype = _mb.dt.np(alloc.dtype)
            out_avals.append(jax.core.ShapedArray(shape, dtype))
            zero_outs.append(np.zeros(shape, dtype))
    n_params = len(in_names)
    all_in_names = in_names + out_names

    def _body(*args):
        return tuple(
            _bass_exec_p.bind(
                *args,
                out_avals=tuple(out_avals),
                in_names=tuple(all_in_names),
                out_names=tuple(out_names),
                lowering_input_output_aliases=(),
                sim_require_finite=True,
                sim_require_nnan=True,
                nc=nc,
            )
        )

    devices = jax.devices()[:NCORES]
    mesh = Mesh(np.asarray(devices), ("core",))
    nio = n_params + len(out_names)
    fn = jax.jit(
        _shard_map(
            _body,
            mesh=mesh,
            in_specs=(PartitionSpec("core"),) * nio,
            out_specs=(PartitionSpec("core"),) * len(out_names),
            check_rep=False,
        )
    )
    concat_in = [
        np.concatenate([np.asarray(in_maps[c][nm]) for c in range(NCORES)], axis=0)
        for nm in in_names
    ] + [np.concatenate([z] * NCORES, axis=0) for z in zero_outs]
    sharding = jax.sharding.NamedSharding(mesh, PartitionSpec("core"))
    handles = [jax.device_put(a, sharding) for a in concat_in]
    best = None
    for _ in range(reps):
        t0 = time.perf_counter()
        outs = fn(*handles)
        jax.block_until_ready(outs)
        dt = time.perf_counter() - t0
        if best is None or dt < best:
            best = dt
    return best * 1e9


def kernel(x, edge_index, W1, b1, W2, b2):
    in_maps, inv_rows, sec_round_w = _prep_inputs(x, edge_index, W1, b1, W2, b2)
    nc = _get_program(sec_round_w)
    res = bass_utils.run_bass_kernel_spmd(
        nc, in_maps, core_ids=list(range(NCORES))
    )
    outs = np.concatenate(
        [res.results[c]["out"] for c in range(NCORES)], axis=0
    )  # [NCORES*SHARD_PAD, OUT_C]
    return np.ascontiguousarray(outs[inv_rows]).astype(np.float32)



# revision 6
# speedup vs baseline: 4.2572x; 4.2572x over previous
"""GCN encoder (2-layer GCNConv) on 8 Trainium2 NeuronCores.

Strategy (dest-sharded graph parallel):
  - Destinations sharded by node range across 8 cores (12500 each).
  - Gathers use InstDMAGatherAnt (dma_gather): ~8192 int16 indices per
    instruction (0.34 ns/descriptor on the Pool Q7, vs ~1 us per 128-row
    indirect_dma_start; multi-offset indirect DMA is broken in HW ucode).
    int16 limits a gather to <32768 table rows, so the replicated node table
    is split into 4 sections (2 cores each).
  - Per (core, section) ELL: dests sorted by per-section in-degree k_b
    descending; round t holds the t-th section-b edge of each dest with
    k_b > t, so round t writes a contiguous prefix of that section's
    accumulator and the scatter-add becomes a contiguous DVE tensor_add.
    The 4 per-section accumulators (each in its own dest order) are merged
    into canonical order by four 12544-index dma_gather permutes.
  - dma_gather requires a 256B-multiple source row stride: the bf16 u-tables
    are AllGather'ed COMPACT (100352 x 16, 3.2 MB) and locally expanded to a
    256B-strided copy (strided HWDGE DMA); only 32B/row is read.
  - Both layers aggregate 16-wide features: layer 2 aggregates BEFORE the
    16x64 transform (aggregation commutes with right-multiplication by W2).

out = D^-1/2 (A+I) D^-1/2 relu(D^-1/2 (A+I) D^-1/2 X W1 + b1) W2 + b2
with u = h * dinv:  s[c] = sum_{e: col=c} u[row_e] + u[c];  out_h = s * dinv + b
"""

import math
import sys

import numpy as np

if "/opt/trn_rl_repo" not in sys.path:
    sys.path.insert(0, "/opt/trn_rl_repo")

import concourse.bacc as bacc
import concourse.bass as bass
import concourse.mybir as mybir
import concourse.tile as tile
from concourse import bass_utils
from concourse.masks import make_identity

# ---------------------------------------------------------------- constants
N = 100000
E = 3200000
IN_C, HID, OUT_C = 128, 16, 64
NCORES = 8
SHARD = N // NCORES            # 12500 real dests per core
P = 128
QCH = (SHARD + P - 1) // P     # 98 column-chunks of 128 ranks
SHARD_PAD = QCH * P            # 12544
SFREE = QCH * HID              # 1568 f32 per partition for s
TABLE_ROWS = NCORES * SHARD_PAD  # 100352 rows in the replicated u table
NSEC = 4                       # table sections (int16 gather-index range)
NQUEUES = 4                    # SWDGE queues (ucode MAX_SWDGE_QUEUES)
SEC_ROWS = TABLE_ROWS // NSEC  # 25088 rows (2 cores) per section
PAD_DEG = 1.0e30               # huge degree for pad ranks -> dinv ~ 1e-15
GCOLS = 64                     # slot columns per dma_gather (8192 descs)
PADROW = 84 * QCH + 97         # local p-major row of pad rank 12500 (u = 0)
MCOLS = SHARD_PAD // 16        # 784 wrapped idx columns per merge gather

F32 = mybir.dt.float32
BF16 = mybir.dt.bfloat16
I32 = mybir.dt.int32
I16 = mybir.dt.int16


def _table_row_local(rank):
    """p-major row of a rank within its core block (matches the staging DMA
    layout '(p q) f -> p (q f)')."""
    return (rank % P) * QCH + rank // P


def _round_profile_sec():
    """Static fallback per-(section, round) widths; the exact per-input
    profile is computed in prep. Per-section in-degree ~ Poisson(E/N/NSEC)
    with dests sorted by it (tight prefix)."""
    lam_b = E / N / NSEC
    R_MAX = 80
    W = []
    for t in range(R_MAX):
        pmf = math.exp(-lam_b)
        cdf = pmf
        for k in range(1, t + 1):
            pmf = pmf * lam_b / k
            cdf += pmf
        q = max(1.0 - cdf, 0.0)
        if SHARD * q < 1e-9 and t > lam_b:
            break
        nt = SHARD * q
        sig = math.sqrt(max(SHARD * q * (1.0 - q), 0.0))
        w = int(math.ceil((nt + 8.0 * sig + 64.0) / P))
        W.append(max(1, min(QCH, w)))
    W += [1] * 6
    return [list(W) for _ in range(NSEC)]


def _pack_runs(runs, bcols):
    """Pack (local_col, width) runs into blocks of <= bcols columns, splitting
    wide runs. Returns list of (block_col_start, ncols, [(lc, j0, w)...]) with
    lc relative to block start and j0 the piece's chunk offset within its
    round."""
    blocks = []
    cur, c0, curw = [], None, 0
    for lc, w in runs:
        j0 = 0
        while w > 0:
            take = min(w, bcols - curw)
            if c0 is None:
                c0 = lc
            cur.append((lc - c0, j0, take))
            curw += take
            lc += take
            j0 += take
            w -= take
            if curw >= bcols:
                blocks.append((c0, curw, cur))
                cur, c0, curw = [], None, 0
    if cur:
        blocks.append((c0, curw, cur))
    return blocks


def _dma_gather(eng, out_ap, in_ap, idxs_ap, num_idxs, elem_size, elem_step,
                single_packet=True, queue_num=0):
    """InstDMAGatherAnt, mirroring bass BassGpSimd.dma_gather but allowing
    elem_size < 256B (the encoding only requires the row STRIDE, elem_step,
    to be a 256B multiple; verified on HW). idxs must be int16, wrapped
    [16, n/16] and replicated across the 8 GPSIMD stripes (128 partitions)."""
    dt_sz = mybir.dt.size(in_ap.dtype)
    stride_bytes = elem_step * dt_sz
    assert stride_bytes % 256 == 0
    _in_ap = eng.lower_ap_dma(in_ap, for_custom_bir_dma=True)
    _idxs_ap = eng.lower_ap(idxs_ap)
    _out_ap = eng.lower_ap(out_ap)
    return eng.add_instruction(
        mybir.InstDMAGatherAnt(
            name=eng.bass.get_next_instruction_name(),
            ins=[*_in_ap, _idxs_ap, eng.lower_val_access(eng.to_reg(num_idxs))],
            outs=[_out_ap],
            transpose=False,
            num_idxs=num_idxs,
            elem_size=elem_size,
            stride_bytes_256=stride_bytes // 256,
            gen_mode=0,
            single_packet=single_packet,
            queue_num=queue_num,
            sbuf_tokens_per_rank=0,
            sbuf_free_dim_per_rank=0,
            sbuf_free_dim_pad_per_rank=0,
            sbuf_byte_offset=0,
        )
    )


# ---------------------------------------------------------------- device code
def _build_program(sec_round_w=None):
    """sec_round_w: list of NSEC lists of per-round column widths."""
    import os
    _skip_ag = bool(os.environ.get("SKIP_AG"))
    _skip_agg = bool(os.environ.get("SKIP_AGG"))
    _skip_merge = bool(os.environ.get("SKIP_MERGE"))
    _skip_out = bool(os.environ.get("SKIP_OUT"))
    if sec_round_w is None:
        sec_round_w = _round_profile_sec()
    sec_cols = [sum(w) for w in sec_round_w]
    sec_runs = []
    for b in range(NSEC):
        runs, pos = [], 0
        for w in sec_round_w[b]:
            runs.append((pos, w))
            pos += w
        sec_runs.append(runs)

    nc = bacc.Bacc(
        "TRN2",
        target_bir_lowering=False,
        debug=False,
        num_devices=NCORES,
        enable_partition_id=False,
        num_swdge_queues=4,
    )
    xT = nc.dram_tensor("xT", [P, SHARD_PAD], BF16, kind="ExternalInput")
    deg_in = nc.dram_tensor("deg", [P, QCH], F32, kind="ExternalInput")
    # wrapped int16 gather indices [16, 8*cols_b per section]; both layers
    # share one copy (identical slot->source mapping)
    offs_in = nc.dram_tensor(
        "offs", [16, 8 * sum(sec_cols)], I16, kind="ExternalInput"
    )
    # wrapped int16 merge-permute indices, MCOLS wrapped cols per section
    midx_in = nc.dram_tensor(
        "midx", [16, NSEC * MCOLS], I16, kind="ExternalInput"
    )
    w1_in = nc.dram_tensor("W1", [IN_C, HID], BF16, kind="ExternalInput")
    w2_in = nc.dram_tensor("W2", [HID, OUT_C], F32, kind="ExternalInput")
    b1_in = nc.dram_tensor("b1", [P, HID], F32, kind="ExternalInput")
    b2_in = nc.dram_tensor("b2", [P, OUT_C], F32, kind="ExternalInput")
    out_d = nc.dram_tensor("out", [SHARD_PAD, OUT_C], F32, kind="ExternalOutput")

    import os as _os
    _gb = int(_os.environ.get("GBUFS", "6"))
    _mb = int(_os.environ.get("MBUFS", "2"))
    with tile.TileContext(nc) as tc:
        with (
            tc.tile_pool(name="const", bufs=1) as cpool,
            tc.tile_pool(name="offs", bufs=2) as opool,
            tc.tile_pool(name="gath", bufs=_gb) as gpool,
            tc.tile_pool(name="merge", bufs=_mb) as mpool,
            tc.tile_pool(name="psum", bufs=3, space="PSUM") as ppool,
            tc.tile_pool(name="psumT", bufs=3, space="PSUM") as ptpool,
            tc.tile_pool(name="dram", bufs=1, space="DRAM") as dpool,
        ):
            # ---- load constants / inputs
            w1_sb = cpool.tile([IN_C, HID], BF16, name="w1_sb")
            w2_sb = cpool.tile([HID, OUT_C], F32, name="w2_sb")
            b1_sb = cpool.tile([P, HID], F32, name="b1_sb")
            b2_sb = cpool.tile([P, OUT_C], F32, name="b2_sb")
            ident = cpool.tile([P, P], F32, name="ident")
            deg_sb = cpool.tile([P, QCH], F32, name="deg_sb")
            dinv = cpool.tile([P, QCH], F32, name="dinv")
            midx_sb = cpool.tile([P, NSEC * MCOLS], I16, name="midx_sb")
            xT_sb = cpool.tile([P, SHARD_PAD], BF16, name="xT_sb")
            u_own = cpool.tile([P, SFREE], BF16, name="u_own")
            u2_own = cpool.tile([P, SFREE], BF16, name="u2_own")
            s_acc = cpool.tile([P, SFREE], F32, name="s_acc")
            v_sb = cpool.tile([P, SFREE], F32, name="v_sb")
            acc = [
                cpool.tile([P, SFREE], F32, name=f"acc{b}") for b in range(NSEC)
            ]
            out_sb = cpool.tile([P, QCH * OUT_C], F32, name="out_sb")

            nc.sync.dma_start(out=w1_sb[:], in_=w1_in[:])
            nc.sync.dma_start(out=w2_sb[:], in_=w2_in[:])
            nc.sync.dma_start(out=b1_sb[:], in_=b1_in[:])
            nc.sync.dma_start(out=b2_sb[:], in_=b2_in[:])
            nc.sync.dma_start(out=deg_sb[:], in_=deg_in[:])
            nc.sync.dma_start(out=xT_sb[:], in_=xT[:])
            msrc = midx_in[:]
            nc.sync.dma_start(
                out=midx_sb[:],
                in_=bass.AP(msrc.tensor, msrc.offset, [[0, 8]] + msrc.ap),
            )
            make_identity(nc, ident[:])

            nc.vector.reciprocal(dinv[:], deg_sb[:])
            nc.scalar.activation(dinv[:], dinv[:], mybir.ActivationFunctionType.Sqrt)

            def dinv16():
                a = dinv[:]
                return bass.AP(a.tensor, a.offset, [a.ap[0], a.ap[1], [0, HID]])

            def b16(t, f):
                a = t[:]
                return bass.AP(a.tensor, a.offset, [a.ap[0], [0, QCH], [1, f]])

            def shaped(ap):
                return ap.rearrange("p (q f) -> p q f", f=HID)

            dram_u1own = dpool.tile([SHARD_PAD, HID], BF16, name="dram_u1own")
            dram_u2own = dpool.tile([SHARD_PAD, HID], BF16, name="dram_u2own")
            u1_tab = dpool.tile(
                [TABLE_ROWS, HID], BF16, name="u1_tab", addr_space="Shared"
            )
            u2_tab = dpool.tile(
                [TABLE_ROWS, HID], BF16, name="u2_tab", addr_space="Shared"
            )
            # 256B-strided gather copies (only first 16 of 128 cols written)
            u1_pad = dpool.tile([TABLE_ROWS, P], BF16, name="u1_pad")
            u2_pad = dpool.tile([TABLE_ROWS, P], BF16, name="u2_pad")
            # 256B-strided per-section accumulator stagings (f32, 64-elem rows)
            dram_acc = [
                [
                    dpool.tile([SHARD_PAD, 64], F32, name=f"dram_acc{li}{b}")
                    for b in range(NSEC)
                ]
                for li in range(2)
            ]

            # ---- layer-1 transform: u1 = (x @ W1) * dinv (bf16), chunk-wise
            for q in range(QCH):
                pt = ppool.tile([P, HID], F32, name="mm1", tag="mm")
                nc.tensor.matmul(
                    out=pt[:],
                    lhsT=xT_sb[:, q * P : (q + 1) * P],
                    rhs=w1_sb[:],
                    start=True,
                    stop=True,
                )
                nc.vector.tensor_scalar(
                    out=u_own[:, q * HID : (q + 1) * HID],
                    in0=pt[:],
                    scalar1=dinv[:, q : q + 1],
                    scalar2=None,
                    op0=mybir.AluOpType.mult,
                )

            def stage_ag_expand(u_sb, dram_own, tab, tab_pad):
                # own slice -> DRAM rows (p-major), AllGather compact table,
                # then per-section strided expand to the 256B-stride copy
                nc.sync.dma_start(
                    out=dram_own[:].rearrange("(p q) f -> p (q f)", p=P),
                    in_=u_sb[:],
                )
                if not _skip_ag:
                    nc.gpsimd.collective_compute(
                        "AllGather",
                        mybir.AluOpType.bypass,
                        replica_groups=[list(range(NCORES))],
                        ins=[dram_own.opt()],
                        outs=[tab.opt()],
                    )
                for b in range(NSEC):
                    r0 = b * SEC_ROWS
                    nc.sync.dma_start(
                        out=tab_pad[r0 : r0 + SEC_ROWS, 0:HID],
                        in_=tab[r0 : r0 + SEC_ROWS, :],
                    )

            stage_ag_expand(u_own, dram_u1own, u1_tab, u1_pad)

            # ---- aggregation of one layer from the padded table
            qn_ctr = [0]

            def next_q():
                q = qn_ctr[0]
                qn_ctr[0] = (q + 1) % NQUEUES
                return q

            def aggregate(tab_pad, sacc, li):
                nc.vector.memset(sacc[:], 0.0)
                MW = 512  # wrapped cols per merge sub-gather (8192 idxs)

                def merge(b):
                    if _skip_merge:
                        return
                    # sacc += perm_b(acc_b) via staged-DRAM dma_gather
                    mg = mpool.tile([P, SFREE], F32, name="mg", tag="mg")
                    for m0 in range(0, MCOLS, MW):
                        mw = min(MW, MCOLS - m0)
                        _dma_gather(
                            nc.gpsimd,
                            out_ap=mg[
                                :, (m0 // 8) * HID : ((m0 + mw) // 8) * HID
                            ].rearrange("p (c e) -> p c e", e=HID),
                            in_ap=dram_acc[li][b][:, 0:HID],
                            idxs_ap=midx_sb[
                                :, b * MCOLS + m0 : b * MCOLS + m0 + mw
                            ],
                            num_idxs=mw * 16,
                            elem_size=HID,
                            elem_step=64,
                            single_packet=False,
                            queue_num=next_q(),
                        )
                    nc.vector.tensor_tensor(
                        out=sacc[:], in0=sacc[:], in1=mg[:],
                        op=mybir.AluOpType.add,
                    )

                col_base = 0
                for b in range(NSEC):
                    cols_b = sec_cols[b]
                    nc.vector.memset(acc[b][:], 0.0)
                    # stream section idxs: broadcast [16, 8*cols] to 128 parts
                    ob = opool.tile([P, 8 * cols_b], I16, name="ob", tag="ob")
                    src = offs_in[:, 8 * col_base : 8 * (col_base + cols_b)]
                    bsrc = bass.AP(src.tensor, src.offset, [[0, 8]] + src.ap)
                    nc.sync.dma_start(out=ob[:], in_=bsrc)
                    r0 = b * SEC_ROWS
                    for c0, ncols, bruns in (
                        [] if _skip_agg else _pack_runs(sec_runs[b], GCOLS)
                    ):
                        g = gpool.tile(
                            [P, GCOLS * HID], BF16, name="gbuf", tag="gbuf"
                        )
                        _dma_gather(
                            nc.gpsimd,
                            out_ap=g[:, : ncols * HID].rearrange(
                                "p (c e) -> p c e", e=HID
                            ),
                            in_ap=tab_pad[r0 : r0 + SEC_ROWS, 0:HID],
                            idxs_ap=ob[:, 8 * c0 : 8 * (c0 + ncols)],
                            num_idxs=ncols * P,
                            elem_size=HID,
                            elem_step=P,
                            single_packet=False,
                            queue_num=next_q(),
                        )
                        for lc, j0, w in bruns:
                            nc.vector.tensor_tensor(
                                out=acc[b][:, j0 * HID : (j0 + w) * HID],
                                in0=acc[b][:, j0 * HID : (j0 + w) * HID],
                                in1=g[:, lc * HID : (lc + w) * HID],
                                op=mybir.AluOpType.add,
                            )
                    col_base += cols_b
                    # stage acc_b to a 256B-strided DRAM table (p-major rows)
                    da = dram_acc[li][b]
                    nc.sync.dma_start(
                        out=bass.AP(
                            da[:].tensor,
                            da[:].offset,
                            [[64 * QCH, P], [64, QCH], [1, HID]],
                        ),
                        in_=acc[b][:],
                    )
                    # merge one section late so its wait never stalls the
                    # Pool queue; merges 0..2 hide under later sections
                    if b >= 1:
                        merge(b - 1)
                merge(NSEC - 1)

            aggregate(u1_pad, s_acc, 0)

            # self loop + finalize: u2 = relu((s + u1) * dinv + b1) * dinv
            nc.vector.tensor_tensor(
                out=s_acc[:], in0=s_acc[:], in1=u_own[:], op=mybir.AluOpType.add
            )
            nc.vector.tensor_tensor(
                out=shaped(s_acc[:]), in0=shaped(s_acc[:]), in1=dinv16(),
                op=mybir.AluOpType.mult,
            )
            nc.vector.tensor_tensor(
                out=shaped(s_acc[:]), in0=shaped(s_acc[:]), in1=b16(b1_sb, HID),
                op=mybir.AluOpType.add,
            )
            nc.scalar.activation(
                s_acc[:], s_acc[:], mybir.ActivationFunctionType.Relu
            )
            nc.vector.tensor_tensor(
                out=shaped(u2_own[:]), in0=shaped(s_acc[:]), in1=dinv16(),
                op=mybir.AluOpType.mult,
            )

            stage_ag_expand(u2_own, dram_u2own, u2_tab, u2_pad)

            # ---- layer-2 aggregation into v, then out = (v*dinv) @ W2 + b2
            aggregate(u2_pad, v_sb, 1)
            nc.vector.tensor_tensor(
                out=v_sb[:], in0=v_sb[:], in1=u2_own[:], op=mybir.AluOpType.add
            )
            nc.vector.tensor_tensor(
                out=shaped(v_sb[:]), in0=shaped(v_sb[:]), in1=dinv16(),
                op=mybir.AluOpType.mult,
            )

            for q in range(0 if not _skip_out else QCH, QCH):
                ptt = ptpool.tile([HID, P], F32, name="vT_ps", tag="vT_ps")
                nc.tensor.transpose(
                    out=ptt[:],
                    in_=v_sb[:, q * HID : (q + 1) * HID],
                    identity=ident[:],
                )
                vT = gpool.tile([HID, P], F32, name="vT_sb", tag="vT_sb")
                nc.vector.tensor_copy(out=vT[:], in_=ptt[:])
                po = ppool.tile([P, OUT_C], F32, name="mm2", tag="mm")
                nc.tensor.matmul(
                    out=po[:], lhsT=vT[:], rhs=w2_sb[:], start=True, stop=True
                )
                nc.vector.tensor_tensor(
                    out=out_sb[:, q * OUT_C : (q + 1) * OUT_C],
                    in0=po[:],
                    in1=b2_sb[:],
                    op=mybir.AluOpType.add,
                )

            nc.sync.dma_start(
                out=out_d[:].rearrange("(p q) f -> p (q f)", p=P),
                in_=out_sb[:],
            )

    nc.compile()
    return nc


_NC_CACHE = {}


def _get_program(sec_round_w=None):
    key = (
        tuple(tuple(w) for w in sec_round_w)
        if sec_round_w is not None
        else None
    )
    if key not in _NC_CACHE:
        _NC_CACHE[key] = _build_program(sec_round_w)
    return _NC_CACHE[key]


# ---------------------------------------------------------------- host prep
def _prep_inputs(x, edge_index, W1, b1, W2, b2):
    """Pure index preprocessing + layout (sharding). Returns in_maps, the
    inverse row permutation for unsharding, and the per-section round
    profile."""
    import ml_dtypes

    x = np.asarray(x, dtype=np.float32)
    row = np.asarray(edge_index[0], dtype=np.int64)
    col = np.asarray(edge_index[1], dtype=np.int64)
    W1 = np.asarray(W1, dtype=np.float32)
    W2 = np.asarray(W2, dtype=np.float32)
    b1 = np.asarray(b1, dtype=np.float32).reshape(-1)
    b2 = np.asarray(b2, dtype=np.float32).reshape(-1)

    indeg = np.bincount(col, minlength=N).astype(np.int64)  # excl self loop
    deg = (indeg + 1).astype(np.float32)

    # canonical per-core rank: own range sorted by total in-degree descending
    rank = np.empty(N, dtype=np.int64)
    node_of_rank = np.empty((NCORES, SHARD_PAD), dtype=np.int64)
    for c in range(NCORES):
        nodes = np.arange(c * SHARD, (c + 1) * SHARD)
        order = np.argsort(-indeg[nodes], kind="stable")
        rank[nodes[order]] = np.arange(SHARD)
        node_of_rank[c, :SHARD] = nodes[order]
        node_of_rank[c, SHARD:] = -1

    core_of = np.arange(N) // SHARD
    # source row within its section's padded table (odd cores upper half)
    local_row = core_of % 2 * SHARD_PAD + _table_row_local(rank)
    sec_of = core_of // 2

    # per-section in-degree per (core, canonical rank)
    dcore_all = col // SHARD
    drank_all = rank[col]
    ssec_all = sec_of[row]
    kb = np.zeros((NSEC, NCORES, SHARD_PAD), dtype=np.int32)
    np.add.at(kb, (ssec_all, dcore_all, drank_all), 1)

    # per-(core, section) dest order: sort by k_b descending; srank = position
    srank = np.empty((NSEC, NCORES, SHARD_PAD), dtype=np.int64)
    sorder = np.empty((NSEC, NCORES, SHARD_PAD), dtype=np.int64)
    for b in range(NSEC):
        for c in range(NCORES):
            o = np.argsort(-kb[b, c], kind="stable")
            sorder[b, c] = o
            srank[b, c, o] = np.arange(SHARD_PAD)

    # exact per-section round profile over the per-section sort (tight):
    # W^b_t = max over cores of ceil(#{k_b > t}/128)
    sec_round_w = []
    for b in range(NSEC):
        maxk = int(kb[b].max())
        wlist = []
        for t in range(maxk):
            wt = 1
            for c in range(NCORES):
                n_tc = int(np.count_nonzero(kb[b, c] > t))
                wt = max(wt, (n_tc + P - 1) // P)
            wlist.append(wt)
        if not wlist:
            wlist = [1]
        sec_round_w.append(wlist)

    sec_cols = [sum(w) for w in sec_round_w]
    tot_cols = sum(sec_cols)
    _prep_inputs.pad_frac = tot_cols * P * NCORES / E - 1.0

    # per-edge slot: section srank of dest + within-(dest,section) counter
    ekey = (dcore_all * SHARD_PAD + drank_all) * NSEC + ssec_all
    eorder = np.argsort(ekey, kind="stable")
    ekey_s = ekey[eorder]
    row_s = row[eorder]
    starts = np.searchsorted(ekey_s, np.arange(NCORES * SHARD_PAD * NSEC))
    t_of = np.arange(E) - starts[ekey_s]
    dsec = ekey_s % NSEC
    drank_s = ekey_s // NSEC % SHARD_PAD
    dc_s = ekey_s // (NSEC * SHARD_PAD)
    sr = srank[dsec, dc_s, drank_s]  # per-section rank of the dest
    qq, pp = sr // P, sr % P

    secbase = np.concatenate([[0], np.cumsum(sec_cols)]).astype(np.int64)
    nr_b = np.asarray([len(w) for w in sec_round_w], dtype=np.int64)
    wt_flat = np.concatenate(
        [np.asarray(w + [0], dtype=np.int64) for w in sec_round_w]
    )
    wbase = np.concatenate([[0], np.cumsum(nr_b + 1)]).astype(np.int64)
    cumw_flat = np.concatenate(
        [np.concatenate([[0], np.cumsum(sec_round_w[b])[:-1]])
         for b in range(NSEC)]
    ).astype(np.int64)
    cb = np.concatenate([[0], np.cumsum(nr_b)]).astype(np.int64)

    tcl = np.minimum(t_of, nr_b[dsec] - 1)
    ok = (t_of < nr_b[dsec]) & (qq < wt_flat[wbase[dsec] + tcl])
    if not np.all(ok):
        raise RuntimeError("per-section round profile exceeded")
    colpos = cumw_flat[cb[dsec] + t_of] + qq      # column within section
    k_flat = (secbase[dsec] + colpos) * P + pp    # global flat slot index
    offs_all = np.full((NCORES, 16, 8 * tot_cols), PADROW, dtype=np.int16)
    offs_all[dc_s, k_flat % 16, k_flat // 16] = local_row[row_s].astype(
        np.int16
    )

    # merge-permute idxs: for canonical rank r, read acc_b at srank[b, c, r]
    midx_all = np.zeros((NCORES, 16, NSEC * MCOLS), dtype=np.int16)
    for b in range(NSEC):
        for c in range(NCORES):
            src_pos = srank[b, c]  # [SHARD_PAD] canonical rank -> srank
            # gather idx k = canonical rank r; table row = p-major of srank
            vals = _table_row_local(src_pos).astype(np.int16)
            k = np.arange(SHARD_PAD)
            midx_all[c, k % 16, b * MCOLS + k // 16] = vals
    # NOTE: gather k -> out[k%128, k//128] = slot (p=r%128, q=r//128) matches
    # s_acc layout (rank r at [r%128, (r//128)*HID]) when k = r.

    # per-core tensors
    in_maps = []
    b1b = np.broadcast_to(b1, (P, HID)).astype(np.float32).copy()
    b2b = np.broadcast_to(b2, (P, OUT_C)).astype(np.float32).copy()
    W1_bf = W1.astype(ml_dtypes.bfloat16)
    for c in range(NCORES):
        nor = node_of_rank[c]
        deg_pi = np.full(SHARD_PAD, PAD_DEG, dtype=np.float32)
        deg_pi[:SHARD] = deg[nor[:SHARD]]
        deg_sb = deg_pi.reshape(QCH, P).T.copy()
        xT = np.zeros((P, SHARD_PAD), dtype=ml_dtypes.bfloat16)
        xT[:, :SHARD] = x[nor[:SHARD]].T.astype(ml_dtypes.bfloat16)
        in_maps.append(
            {
                "xT": np.ascontiguousarray(xT),
                "deg": np.ascontiguousarray(deg_sb),
                "offs": np.ascontiguousarray(offs_all[c]),
                "midx": np.ascontiguousarray(midx_all[c]),
                "W1": W1_bf,
                "W2": W2,
                "b1": b1b,
                "b2": b2b,
            }
        )

    # unshard: out row of node (concat over cores) = core*SHARD_PAD + p-major
    inv_rows = core_of * SHARD_PAD + _table_row_local(rank)
    global OFFS_W
    OFFS_W = tot_cols
    return in_maps, inv_rows, sec_round_w


OFFS_W = 0


def _build_floor_probe():
    """Minimal 8-core program for measuring the PJRT dispatch floor."""
    nc = bacc.Bacc("TRN2", target_bir_lowering=False, debug=False,
                   num_devices=NCORES, enable_partition_id=False)
    a = nc.dram_tensor("a", [P, 16], F32, kind="ExternalInput")
    b = nc.dram_tensor("b", [P, 16], F32, kind="ExternalOutput")
    with tile.TileContext(nc) as tc:
        with tc.tile_pool(name="sb", bufs=1) as sb:
            t = sb.tile([P, 16], F32, name="t")
            nc.sync.dma_start(out=t[:], in_=a[:])
            nc.sync.dma_start(out=b[:], in_=t[:])
    nc.compile()
    return nc


def timed_run(in_maps, reps=5, nc=None, round_w=None):
    """Time device execution of the compiled program (PJRT path, inputs
    pre-staged on device). Returns best wall-ns per execution."""
    import time

    import jax
    from jax.sharding import Mesh, PartitionSpec
    from jax.experimental.shard_map import shard_map as _shard_map

    if nc is None:
        nc = _get_program(round_w)
    import concourse.mybir as _mb
    from concourse.bass2jax import _bass_exec_p, install_neuronx_cc_hook

    install_neuronx_cc_hook()
    in_names, out_names, out_avals, zero_outs = [], [], [], []
    for alloc in nc.m.functions[0].allocations:
        if not isinstance(alloc, _mb.MemoryLocationSet):
            continue
        name = alloc.memorylocations[0].name
        if alloc.kind == "ExternalInput":
            in_names.append(name)
        elif alloc.kind == "ExternalOutput":
            out_names.append(name)
            shape = tuple(alloc.tensor_shape)
            dtype = _mb.dt.np(alloc.dtype)
            out_avals.append(jax.core.ShapedArray(shape, dtype))
            zero_outs.append(np.zeros(shape, dtype))
    n_params = len(in_names)
    all_in_names = in_names + out_names

    def _body(*args):
        return tuple(
            _bass_exec_p.bind(
                *args,
                out_avals=tuple(out_avals),
                in_names=tuple(all_in_names),
                out_names=tuple(out_names),
                lowering_input_output_aliases=(),
                sim_require_finite=True,
                sim_require_nnan=True,
                nc=nc,
            )
        )

    devices = jax.devices()[:NCORES]
    mesh = Mesh(np.asarray(devices), ("core",))
    nio = n_params + len(out_names)
    fn = jax.jit(
        _shard_map(
            _body,
            mesh=mesh,
            in_specs=(PartitionSpec("core"),) * nio,
            out_specs=(PartitionSpec("core"),) * len(out_names),
            check_rep=False,
        )
    )
    concat_in = [
        np.concatenate([np.asarray(in_maps[c][nm]) for c in range(NCORES)], axis=0)
        for nm in in_names
    ] + [np.concatenate([z] * NCORES, axis=0) for z in zero_outs]
    sharding = jax.sharding.NamedSharding(mesh, PartitionSpec("core"))
    handles = [jax.device_put(a, sharding) for a in concat_in]
    best = None
    for _ in range(reps):
        t0 = time.perf_counter()
        outs = fn(*handles)
        jax.block_until_ready(outs)
        dt = time.perf_counter() - t0
        if best is None or dt < best:
            best = dt
    return best * 1e9


def kernel(x, edge_index, W1, b1, W2, b2):
    in_maps, inv_rows, sec_round_w = _prep_inputs(x, edge_index, W1, b1, W2, b2)
    nc = _get_program(sec_round_w)
    res = bass_utils.run_bass_kernel_spmd(
        nc, in_maps, core_ids=list(range(NCORES))
    )
    outs = np.concatenate(
        [res.results[c]["out"] for c in range(NCORES)], axis=0
    )  # [NCORES*SHARD_PAD, OUT_C]
    return np.ascontiguousarray(outs[inv_rows]).astype(np.float32)

